# revision 2
# baseline (speedup 1.0000x reference)
"""Trainium2 Bass kernel for nn_C4ByteNibbleVM — bitpacked pipeline.

The reference "soft VM" computes, per 32-bit word (4 bytes, one-hot f32
encoded), out = onehot(((a + b) mod 2^32) ^ a) bytewise with a ripple
carry.  With exact one-hot inputs every softmax in the reference is
saturated, so the reference output equals the exact integer result.

Host <-> device interface uses a bitpacked (bijective, per-element)
re-encoding of the one-hots: each 256-wide f32 one-hot row becomes a
256-bit mask (16 uint16 lanes, little-endian bit order).  All actual
computation — locating the hot bit (argmax), the 4-byte ripple-carry
add, the xor, and re-expanding to a one-hot mask — happens on device:

  1. ScalarE copies the u16 mask lanes (value 2^m or 0) to bf16; the
     bf16 bit pattern is (127+m)<<7 for nonzero lanes, 0 for zero
     lanes (low 7 bits clear).  Both operands land in one tile.
  2. DVE ors the lane id j (0..15, stride-0 broadcast AP) into the low
     bits, then max-folds the 16 lanes of each segment in one shared
     tree: e = ((127+m)<<7)|j.
  3. byte value c = 16*j + m = ((e&15)<<4) + (e>>7) - 127.
  4. u16 ripple-carry add (fused is_ge+add chain) + xor -> x.
  5. expansion in a lane-plane layout: plane j holds d = x - 16j
     (log-doubling with contiguous subtracts), then ONE tensor_tensor
     1<<d — hardware shifts by amounts outside [0,15] (including
     wrapped-negative u16) produce exactly 0, so plane j is 2^(x&15)
     where x>>4==j and 0 elsewhere.  The host un-permutes the
     plane-major layout (a free numpy transpose).

Engine notes (HW-validated): tensor_tensor and all bitwise ops are
DVE-only; GPSIMD compute costs ~4us per instruction on HW (avoid);
stride-0 broadcast APs run at full DVE speed; ScalarE does the int ->
bf16 converting copies and a DMA queue.

Data parallel over words: 8192 words per core x 8 cores.
Layout: word w = p*64 + s (p = SBUF partition, s = 0..63) so every
DMA moves 8 KB contiguous per partition.
"""

import numpy as np

import concourse.bacc as bacc
import concourse.mybir as mybir
from concourse.tile import TileContext
from concourse import bass_utils

B = 65536
NCORES = 8
BLOC = B // NCORES          # words per core
S = BLOC // 128             # words per partition (64)
FD = S * 64                 # u16 lanes per partition (4096)
NSEG = S * 4                # one-hot segments per partition (256)

F32 = mybir.dt.float32
BF16 = mybir.dt.bfloat16
U16 = mybir.dt.uint16
I16 = mybir.dt.int16
AX = mybir.AxisListType
OP = mybir.AluOpType
AF = mybir.ActivationFunctionType


def build_kernel(n_words=BLOC, w=None, reps=1):
    """Per-core Bass module. n_words words; w ignored (kept for test.py)."""
    s = n_words // 128
    fd = s * 64
    nseg = s * 4

    nc = bacc.Bacc("TRN2", target_bir_lowering=False, debug=False)
    # a and b stacked: rows [0:n) = a, [n:2n) = b; one DMA, one convert
    ab_d = nc.dram_tensor("ab", [2 * n_words, 64], U16, kind="ExternalInput")
    # plane-major output: [p, j, s, seg] flattened; host un-permutes
    y_d = nc.dram_tensor("y", [128, fd], U16, kind="ExternalOutput")

    ab_v = ab_d[:].rearrange("(t p s) c -> p t s c", t=2, p=128)

    with TileContext(nc) as tc:
        with (
            tc.tile_pool(name="cst", bufs=1) as cst,
            tc.tile_pool(name="ld", bufs=2) as ld,
            tc.tile_pool(name="wk", bufs=2) as wk,
            tc.tile_pool(name="idx", bufs=2) as idxp,
            tc.tile_pool(name="out", bufs=2) as outp,
        ):
            # lane-id constant 0..15 (stride-0 broadcast in the or)
            jc = cst.tile([128, 16], U16)
            nc.gpsimd.iota(jc[:], pattern=[[1, 16]], base=0,
                           channel_multiplier=0)
            ones = cst.tile([128, fd], U16)
            nc.gpsimd.memset(ones[:], 1)

            prev_out = None
            for _ in range(reps):
                in_ab = ld.tile([128, 2 * fd], U16, tag="inab")
                nc.sync.dma_start(
                    in_ab[:].rearrange("p (t s c) -> p t s c", t=2, c=64),
                    ab_v)
                # previous rep's store, software-pipelined so the ACT
                # queue never stalls waiting for this rep's output
                if prev_out is not None:
                    nc.scalar.dma_start(y_d[:], prev_out[:])

                # int -> bf16 converting copy on ScalarE (frees DVE);
                # both operands in one tile so or+folds+decode run fused
                fb = wk.tile([128, 2 * fd], BF16, tag="fb")
                nc.scalar.activation(fb[:], in_ab[:], AF.Copy)

                t2 = wk.tile([128, 2 * fd], U16, tag="t2")
                nc.vector.tensor_tensor(
                    t2[:].rearrange("p (g c) -> p g c", c=16),
                    fb[:].bitcast(U16).rearrange("p (g c) -> p g c", c=16),
                    jc[:].unsqueeze(1).broadcast_to([128, 2 * nseg, 16]),
                    OP.bitwise_or)
                cur = t2[:].rearrange("p (g c) -> p g c", c=16)
                width = 16
                while width > 1:
                    width //= 2
                    nxt_t = wk.tile([128, 2 * nseg * width], U16,
                                    tag=f"f{width}")
                    nxt = nxt_t[:].rearrange("p (g c) -> p g c", c=width)
                    nc.vector.tensor_tensor(
                        nxt, cur[:, :, 0:width],
                        cur[:, :, width:2 * width], OP.max)
                    cur = nxt
                e_ab = nxt_t

                # byte value c = ((e&15)<<4) + (e>>7) - 127, both halves
                # (bitwise ts ops cannot cast or mix with arith ops)
                cA = idxp.tile([128, 2 * nseg], U16, tag="cA")
                nc.vector.tensor_scalar(
                    cA[:], e_ab[:], 15, 4, OP.bitwise_and,
                    OP.logical_shift_left)
                cB = idxp.tile([128, 2 * nseg], U16, tag="cB")
                nc.vector.tensor_scalar(
                    cB[:], e_ab[:], 7, None, OP.logical_shift_right)
                ct = idxp.tile([128, 2 * nseg], U16, tag="ct")
                nc.vector.tensor_tensor(ct[:], cA[:], cB[:], OP.add)
                c = idxp.tile([128, 2 * nseg], U16, tag="c")
                nc.vector.tensor_scalar(c[:], ct[:], 127, None, OP.subtract)
                ca, cb = c[:, 0:nseg], c[:, nseg:2 * nseg]

                # ripple-carry add: tf_i += (tf_{i-1} >= 256), then &255
                def bslice(ap, i):
                    return ap.rearrange("p (s i) -> p i s", i=4)[:, i:i + 1, :]

                tf = idxp.tile([128, nseg], U16, tag="tf")
                nc.vector.tensor_tensor(tf[:], ca, cb, OP.add)
                for i in range(1, 4):
                    nc.vector.scalar_tensor_tensor(
                        bslice(tf[:], i), bslice(tf[:], i - 1), 256,
                        bslice(tf[:], i), OP.is_ge, OP.add)
                csum = idxp.tile([128, nseg], U16, tag="cs")
                nc.vector.tensor_scalar(
                    csum[:], tf[:], 255, None, OP.bitwise_and)
                x = idxp.tile([128, nseg], U16, tag="x")
                nc.vector.tensor_tensor(x[:], csum[:], ca, OP.bitwise_xor)

                # expansion: plane j = x - 16j (log-doubling, contiguous).
                # d must be SIGNED: the DVE's fp32->u16 output conversion
                # clamps negatives to 0 on HW (the sim wraps), which would
                # turn planes above hi into 1<<0.  i16 keeps the negatives;
                # the u16-bitcast view then feeds the shift, where any
                # amount outside [0,15] (incl. wrapped) yields exactly 0.
                d = outp.tile([128, fd], I16, tag="d")
                nc.vector.tensor_copy(d[:, 0:nseg], x[:])
                span = nseg
                while span < fd:
                    nc.vector.tensor_scalar(
                        d[:, span:2 * span], d[:, 0:span],
                        16 * (span // nseg), None, OP.subtract)
                    span *= 2
                out_t = outp.tile([128, fd], U16, tag="o")
                nc.vector.tensor_tensor(
                    out_t[:], ones[:], d[:].bitcast(U16),
                    OP.logical_shift_left)
                prev_out = out_t
            nc.scalar.dma_start(y_d[:], prev_out[:])

    nc.compile()
    return nc


_CACHED = {}


def _get_kernel():
    if "k" not in _CACHED:
        _CACHED["k"] = build_kernel()
    return _CACHED["k"]


def _pack(x):
    """f32 one-hot [n,4,256] -> u16 bitmask [n,64]."""
    bits = np.packbits(x != 0, axis=-1, bitorder="little")  # [n,4,32] u8
    return np.ascontiguousarray(bits.reshape(x.shape[0], 128)).view(np.uint16)


def _unpermute(y_raw, n_words=BLOC):
    """Device plane-major [128, fd] -> word-major bitmask [n_words, 64]."""
    s = n_words // 128
    y = y_raw.reshape(128, 16, s, 4)          # p, j, s, seg
    y = y.transpose(0, 2, 3, 1)               # p, s, seg, j
    return np.ascontiguousarray(y.reshape(n_words, 64))


def _unpack(y):
    """u16 bitmask [n,64] -> f32 one-hot [n,4,256]."""
    y8 = np.ascontiguousarray(y).view(np.uint8)             # [n,128]
    bits = np.unpackbits(y8, axis=1, bitorder="little")     # [n,1024]
    return bits.reshape(y.shape[0], 4, 256).astype(np.float32)


def kernel(**inputs):
    a = _pack(np.asarray(inputs["a_bytes"]))
    b = _pack(np.asarray(inputs["b_bytes"]))
    nc = _get_kernel()
    in_maps = [
        {"ab": np.concatenate([a[c * BLOC:(c + 1) * BLOC],
                               b[c * BLOC:(c + 1) * BLOC]])}
        for c in range(NCORES)
    ]
    res = bass_utils.run_bass_kernel_spmd(nc, in_maps, core_ids=list(range(NCORES)))
    out = np.concatenate(
        [_unpermute(res.results[c]["y"]) for c in range(NCORES)], axis=0)
    return _unpack(out)


# revision 3
# speedup vs baseline: 1.2852x; 1.2852x over previous
"""Trainium2 Bass kernel for nn_C4ByteNibbleVM — bitpacked pipeline.

The reference "soft VM" computes, per 32-bit word (4 bytes, one-hot f32
encoded), out = onehot(((a + b) mod 2^32) ^ a) bytewise with a ripple
carry.  With exact one-hot inputs every softmax in the reference is
saturated, so the reference output equals the exact integer result.

Host <-> device interface uses a bitpacked (bijective, per-element)
re-encoding of the one-hots: each 256-wide f32 one-hot row becomes a
256-bit mask (16 uint16 lanes, little-endian bit order).  All actual
computation — locating the hot bit (argmax), the 4-byte ripple-carry
add, the xor, and re-expanding to a one-hot mask — happens on device:

  1. ScalarE copies the u16 mask lanes (value 2^m or 0) to bf16; the
     bf16 bit pattern is (127+m)<<7 for nonzero lanes, 0 for zero
     lanes (low 7 bits clear).  Both operands land in one tile.
  2. DVE ors the lane id j (0..15, stride-0 broadcast AP) into the low
     bits, then max-folds the 16 lanes of each segment in one shared
     tree: e = ((127+m)<<7)|j.
  3. byte value c = 16*j + m = ((e&15)<<4) + (e>>7) - 127.
  4. u16 ripple-carry add (fused is_ge+add chain) + xor -> x.
  5. expansion in a lane-plane layout: plane j holds d = x - 16j
     (log-doubling with contiguous subtracts), then ONE tensor_tensor
     1<<d — hardware shifts by amounts outside [0,15] (including
     wrapped-negative u16) produce exactly 0, so plane j is 2^(x&15)
     where x>>4==j and 0 elsewhere.  The host un-permutes the
     plane-major layout (a free numpy transpose).

Engine notes (HW-validated): tensor_tensor and all bitwise ops are
DVE-only; GPSIMD compute costs ~4us per instruction on HW (avoid);
stride-0 broadcast APs run at full DVE speed; ScalarE does the int ->
bf16 converting copies and a DMA queue.

Data parallel over words: 8192 words per core x 8 cores.
Layout: word w = p*64 + s (p = SBUF partition, s = 0..63) so every
DMA moves 8 KB contiguous per partition.
"""

import numpy as np

import concourse.bacc as bacc
import concourse.mybir as mybir
from concourse.tile import TileContext
from concourse import bass_utils

B = 65536
NCORES = 8
BLOC = B // NCORES          # words per core
S = BLOC // 128             # words per partition (64)
FD = S * 64                 # u16 lanes per partition (4096)
NSEG = S * 4                # one-hot segments per partition (256)

F32 = mybir.dt.float32
BF16 = mybir.dt.bfloat16
U16 = mybir.dt.uint16
I16 = mybir.dt.int16
AX = mybir.AxisListType
OP = mybir.AluOpType
AF = mybir.ActivationFunctionType


def build_kernel(n_words=BLOC, w=None, reps=1):
    """Per-core Bass module. n_words words; w ignored (kept for test.py)."""
    s = n_words // 128
    fd = s * 64
    nseg = s * 4

    nc = bacc.Bacc("TRN2", target_bir_lowering=False, debug=False)
    # a and b stacked: rows [0:n) = a, [n:2n) = b; one DMA, one convert
    ab_d = nc.dram_tensor("ab", [2 * n_words, 64], U16, kind="ExternalInput")
    # plane-major output: [p, j, s, seg] flattened; host un-permutes
    y_d = nc.dram_tensor("y", [128, fd], U16, kind="ExternalOutput")

    ab_v = ab_d[:].rearrange("(t p s) c -> p t s c", t=2, p=128)

    with TileContext(nc) as tc:
        with (
            tc.tile_pool(name="cst", bufs=1) as cst,
            tc.tile_pool(name="ld", bufs=2) as ld,
            tc.tile_pool(name="wk", bufs=2) as wk,
            tc.tile_pool(name="idx", bufs=2) as idxp,
            tc.tile_pool(name="out", bufs=2) as outp,
        ):
            # lane-id constant 0..15 (stride-0 broadcast in the or)
            jc = cst.tile([128, 16], U16)
            nc.gpsimd.iota(jc[:], pattern=[[1, 16]], base=0,
                           channel_multiplier=0)
            ones = cst.tile([128, fd], U16)
            nc.gpsimd.memset(ones[:], 1)

            prev_out = None
            for _ in range(reps):
                in_ab = ld.tile([128, 2 * fd], U16, tag="inab")
                nc.sync.dma_start(
                    in_ab[:].rearrange("p (t s c) -> p t s c", t=2, c=64),
                    ab_v)
                # previous rep's store, software-pipelined so the ACT
                # queue never stalls waiting for this rep's output
                if prev_out is not None:
                    nc.scalar.dma_start(y_d[:], prev_out[:])

                # int -> bf16 converting copy on ScalarE (frees DVE);
                # both operands in one tile so or+folds+decode run fused
                fb = wk.tile([128, 2 * fd], BF16, tag="fb")
                nc.scalar.activation(fb[:], in_ab[:], AF.Copy)

                t2 = wk.tile([128, 2 * fd], U16, tag="t2")
                nc.vector.tensor_tensor(
                    t2[:].rearrange("p (g c) -> p g c", c=16),
                    fb[:].bitcast(U16).rearrange("p (g c) -> p g c", c=16),
                    jc[:].unsqueeze(1).broadcast_to([128, 2 * nseg, 16]),
                    OP.bitwise_or)
                cur = t2[:].rearrange("p (g c) -> p g c", c=16)
                width = 16
                while width > 1:
                    width //= 2
                    nxt_t = wk.tile([128, 2 * nseg * width], U16,
                                    tag=f"f{width}")
                    nxt = nxt_t[:].rearrange("p (g c) -> p g c", c=width)
                    nc.vector.tensor_tensor(
                        nxt, cur[:, :, 0:width],
                        cur[:, :, width:2 * width], OP.max)
                    cur = nxt
                e_ab = nxt_t

                # byte value c = ((e&15)<<4) + (e>>7) - 127, both halves
                # (bitwise ts ops cannot cast or mix with arith ops)
                cA = idxp.tile([128, 2 * nseg], U16, tag="cA")
                nc.vector.tensor_scalar(
                    cA[:], e_ab[:], 15, 4, OP.bitwise_and,
                    OP.logical_shift_left)
                cB = idxp.tile([128, 2 * nseg], U16, tag="cB")
                nc.vector.tensor_scalar(
                    cB[:], e_ab[:], 7, None, OP.logical_shift_right)
                ct = idxp.tile([128, 2 * nseg], U16, tag="ct")
                nc.vector.tensor_tensor(ct[:], cA[:], cB[:], OP.add)
                c = idxp.tile([128, 2 * nseg], U16, tag="c")
                nc.vector.tensor_scalar(c[:], ct[:], 127, None, OP.subtract)
                ca, cb = c[:, 0:nseg], c[:, nseg:2 * nseg]

                # ripple-carry add: tf_i += (tf_{i-1} >= 256), then &255
                def bslice(ap, i):
                    return ap.rearrange("p (s i) -> p i s", i=4)[:, i:i + 1, :]

                tf = idxp.tile([128, nseg], U16, tag="tf")
                nc.vector.tensor_tensor(tf[:], ca, cb, OP.add)
                for i in range(1, 4):
                    nc.vector.scalar_tensor_tensor(
                        bslice(tf[:], i), bslice(tf[:], i - 1), 256,
                        bslice(tf[:], i), OP.is_ge, OP.add)
                csum = idxp.tile([128, nseg], U16, tag="cs")
                nc.vector.tensor_scalar(
                    csum[:], tf[:], 255, None, OP.bitwise_and)
                x = idxp.tile([128, nseg], U16, tag="x")
                nc.vector.tensor_tensor(x[:], csum[:], ca, OP.bitwise_xor)

                # expansion: plane j = x - 16j (log-doubling, contiguous).
                # d must be SIGNED: the DVE's fp32->u16 output conversion
                # clamps negatives to 0 on HW (the sim wraps), which would
                # turn planes above hi into 1<<0.  i16 keeps the negatives;
                # the u16-bitcast view then feeds the shift, where any
                # amount outside [0,15] (incl. wrapped) yields exactly 0.
                d = outp.tile([128, fd], I16, tag="d")
                nc.vector.tensor_copy(d[:, 0:nseg], x[:])
                span = nseg
                while span < fd:
                    if span < fd // 2:
                        # small doubling steps on ScalarE (bias = -16k)
                        nc.scalar.activation(
                            d[:, span:2 * span], d[:, 0:span], AF.Copy,
                            bias=float(-16 * (span // nseg)))
                    else:
                        nc.vector.tensor_scalar(
                            d[:, span:2 * span], d[:, 0:span],
                            16 * (span // nseg), None, OP.subtract)
                    span *= 2
                out_t = outp.tile([128, fd], U16, tag="o")
                nc.vector.tensor_tensor(
                    out_t[:], ones[:], d[:].bitcast(U16),
                    OP.logical_shift_left)
                prev_out = out_t
            nc.scalar.dma_start(y_d[:], prev_out[:])

    nc.compile()
    return nc


_CACHED = {}


def _get_kernel():
    if "k" not in _CACHED:
        _CACHED["k"] = build_kernel()
    return _CACHED["k"]


def _pack(x):
    """f32 one-hot [n,4,256] -> u16 bitmask [n,64]."""
    bits = np.packbits(x != 0, axis=-1, bitorder="little")  # [n,4,32] u8
    return np.ascontiguousarray(bits.reshape(x.shape[0], 128)).view(np.uint16)


def _unpermute(y_raw, n_words=BLOC):
    """Device plane-major [128, fd] -> word-major bitmask [n_words, 64]."""
    s = n_words // 128
    y = y_raw.reshape(128, 16, s, 4)          # p, j, s, seg
    y = y.transpose(0, 2, 3, 1)               # p, s, seg, j
    return np.ascontiguousarray(y.reshape(n_words, 64))


def _unpack(y):
    """u16 bitmask [n,64] -> f32 one-hot [n,4,256]."""
    y8 = np.ascontiguousarray(y).view(np.uint8)             # [n,128]
    bits = np.unpackbits(y8, axis=1, bitorder="little")     # [n,1024]
    return bits.reshape(y.shape[0], 4, 256).astype(np.float32)


def kernel(**inputs):
    a = _pack(np.asarray(inputs["a_bytes"]))
    b = _pack(np.asarray(inputs["b_bytes"]))
    nc = _get_kernel()
    in_maps = [
        {"ab": np.concatenate([a[c * BLOC:(c + 1) * BLOC],
                               b[c * BLOC:(c + 1) * BLOC]])}
        for c in range(NCORES)
    ]
    res = bass_utils.run_bass_kernel_spmd(nc, in_maps, core_ids=list(range(NCORES)))
    out = np.concatenate(
        [_unpermute(res.results[c]["y"]) for c in range(NCORES)], axis=0)
    return _unpack(out)


# revision 4
# speedup vs baseline: 1.3639x; 1.0612x over previous
"""Trainium2 Bass kernel for nn_C4ByteNibbleVM — bitpacked pipeline.

The reference "soft VM" computes, per 32-bit word (4 bytes, one-hot f32
encoded), out = onehot(((a + b) mod 2^32) ^ a) bytewise with a ripple
carry.  With exact one-hot inputs every softmax in the reference is
saturated, so the reference output equals the exact integer result.

Host <-> device interface uses a bitpacked (bijective, per-element)
re-encoding of the one-hots: each 256-wide f32 one-hot row becomes a
256-bit mask (16 uint16 lanes, little-endian bit order).  All actual
computation — locating the hot bit (argmax), the 4-byte ripple-carry
add, the xor, and re-expanding to a one-hot mask — happens on device:

  1. ScalarE copies the u16 mask lanes (value 2^m or 0) to bf16; the
     bf16 bit pattern is (127+m)<<7 for nonzero lanes, 0 for zero
     lanes (low 7 bits clear).  Both operands land in one tile.
  2. DVE ors the lane id j (0..15, stride-0 broadcast AP) into the low
     bits, then max-folds the 16 lanes of each segment in one shared
     tree: e = ((127+m)<<7)|j.
  3. byte value c = 16*j + m = ((e&15)<<4) + (e>>7) - 127.
  4. u16 ripple-carry add (fused is_ge+add chain) + xor -> x.
  5. expansion in a lane-plane layout: plane j holds d = x - 16j
     (log-doubling with contiguous subtracts), then ONE tensor_tensor
     1<<d — hardware shifts by amounts outside [0,15] (including
     wrapped-negative u16) produce exactly 0, so plane j is 2^(x&15)
     where x>>4==j and 0 elsewhere.  The host un-permutes the
     plane-major layout (a free numpy transpose).

Engine notes (HW-validated): tensor_tensor and all bitwise ops are
DVE-only; GPSIMD compute costs ~4us per instruction on HW (avoid);
stride-0 broadcast APs run at full DVE speed; ScalarE does the int ->
bf16 converting copies and a DMA queue.

Data parallel over words: 8192 words per core x 8 cores.
Layout: word w = p*64 + s (p = SBUF partition, s = 0..63) so every
DMA moves 8 KB contiguous per partition.
"""

import numpy as np

import concourse.bacc as bacc
import concourse.mybir as mybir
from concourse.tile import TileContext
from concourse import bass_utils

B = 65536
NCORES = 8
BLOC = B // NCORES          # words per core
S = BLOC // 128             # words per partition (64)
FD = S * 64                 # u16 lanes per partition (4096)
NSEG = S * 4                # one-hot segments per partition (256)

F32 = mybir.dt.float32
BF16 = mybir.dt.bfloat16
U16 = mybir.dt.uint16
I16 = mybir.dt.int16
AX = mybir.AxisListType
OP = mybir.AluOpType
AF = mybir.ActivationFunctionType


def build_kernel(n_words=BLOC, w=None, reps=1):
    """Per-core Bass module. n_words words; w ignored (kept for test.py)."""
    s = n_words // 128
    fd = s * 64
    nseg = s * 4

    nc = bacc.Bacc("TRN2", target_bir_lowering=False, debug=False)
    # a and b stacked: rows [0:n) = a, [n:2n) = b; one DMA, one convert
    ab_d = nc.dram_tensor("ab", [2 * n_words, 64], U16, kind="ExternalInput")
    # plane-major output: [p, j, s, seg] flattened; host un-permutes
    y_d = nc.dram_tensor("y", [128, fd], U16, kind="ExternalOutput")

    ab_v = ab_d[:].rearrange("(t p s) c -> p t s c", t=2, p=128)

    with TileContext(nc) as tc:
        with (
            tc.tile_pool(name="cst", bufs=1) as cst,
            tc.tile_pool(name="ld", bufs=2) as ld,
            tc.tile_pool(name="wk", bufs=2) as wk,
            tc.tile_pool(name="idx", bufs=2) as idxp,
            tc.tile_pool(name="out", bufs=2) as outp,
        ):
            # lane-id constant 0..15 (stride-0 broadcast in the or)
            jc = cst.tile([128, 16], U16)
            nc.gpsimd.iota(jc[:], pattern=[[1, 16]], base=0,
                           channel_multiplier=0)
            ones = cst.tile([128, fd], U16)
            nc.gpsimd.memset(ones[:], 1)

            prev_out = None
            for _ in range(reps):
                in_ab = ld.tile([128, 2 * fd], U16, tag="inab")
                nc.sync.dma_start(
                    in_ab[:].rearrange("p (t s c) -> p t s c", t=2, c=64),
                    ab_v)
                # previous rep's store, software-pipelined so the ACT
                # queue never stalls waiting for this rep's output
                if prev_out is not None:
                    nc.scalar.dma_start(y_d[:], prev_out[:])

                # int -> bf16 converting copy on ScalarE (frees DVE);
                # both operands in one tile so or+folds+decode run fused
                fb = wk.tile([128, 2 * fd], BF16, tag="fb")
                nc.scalar.activation(fb[:], in_ab[:], AF.Copy)

                t2 = wk.tile([128, 2 * fd], U16, tag="t2")
                nc.vector.tensor_tensor(
                    t2[:].rearrange("p (g c) -> p g c", c=16),
                    fb[:].bitcast(U16).rearrange("p (g c) -> p g c", c=16),
                    jc[:].unsqueeze(1).broadcast_to([128, 2 * nseg, 16]),
                    OP.bitwise_or)
                cur = t2[:].rearrange("p (g c) -> p g c", c=16)
                width = 16
                while width > 1:
                    width //= 2
                    nxt_t = wk.tile([128, 2 * nseg * width], U16,
                                    tag=f"f{width}")
                    nxt = nxt_t[:].rearrange("p (g c) -> p g c", c=width)
                    nc.vector.tensor_tensor(
                        nxt, cur[:, :, 0:width],
                        cur[:, :, width:2 * width], OP.max)
                    cur = nxt
                e_ab = nxt_t

                # byte value c = ((e&15)<<4) + (e>>7) - 127, both halves
                # (bitwise ts ops cannot cast or mix with arith ops)
                cA = idxp.tile([128, 2 * nseg], U16, tag="cA")
                nc.vector.tensor_scalar(
                    cA[:], e_ab[:], 15, 4, OP.bitwise_and,
                    OP.logical_shift_left)
                cB = idxp.tile([128, 2 * nseg], U16, tag="cB")
                nc.vector.tensor_scalar(
                    cB[:], e_ab[:], 7, None, OP.logical_shift_right)
                ct = idxp.tile([128, 2 * nseg], U16, tag="ct")
                nc.vector.tensor_tensor(ct[:], cA[:], cB[:], OP.add)
                # i16 from here on so the xor can write straight into the
                # expansion tile (bitwise ops cannot cast dtypes)
                c = idxp.tile([128, 2 * nseg], I16, tag="c")
                nc.vector.tensor_scalar(c[:], ct[:], 127, None, OP.subtract)
                ca, cb = c[:, 0:nseg], c[:, nseg:2 * nseg]

                # ripple-carry add: tf_i += (tf_{i-1} >= 256), then &255
                def bslice(ap, i):
                    return ap.rearrange("p (s i) -> p i s", i=4)[:, i:i + 1, :]

                tf = idxp.tile([128, nseg], I16, tag="tf")
                nc.vector.tensor_tensor(tf[:], ca, cb, OP.add)
                for i in range(1, 4):
                    nc.vector.scalar_tensor_tensor(
                        bslice(tf[:], i), bslice(tf[:], i - 1), 256,
                        bslice(tf[:], i), OP.is_ge, OP.add)
                csum = idxp.tile([128, nseg], I16, tag="cs")
                nc.vector.tensor_scalar(
                    csum[:], tf[:], 255, None, OP.bitwise_and)

                # expansion: plane j = x - 16j (log-doubling, contiguous).
                # d must be SIGNED: the DVE's fp32->u16 output conversion
                # clamps negatives to 0 on HW (the sim wraps), which would
                # turn planes above hi into 1<<0.  i16 keeps the negatives;
                # the u16-bitcast view then feeds the shift, where any
                # amount outside [0,15] (incl. wrapped) yields exactly 0.
                # The xor writes plane 0 of d directly (x = csum ^ ca).
                d = outp.tile([128, fd], I16, tag="d")
                nc.vector.tensor_tensor(
                    d[:, 0:nseg], csum[:], ca, OP.bitwise_xor)
                span = nseg
                while span < fd:
                    if span < fd // 2:
                        # small doubling steps on ScalarE (bias = -16k)
                        nc.scalar.activation(
                            d[:, span:2 * span], d[:, 0:span], AF.Copy,
                            bias=float(-16 * (span // nseg)))
                    else:
                        nc.vector.tensor_scalar(
                            d[:, span:2 * span], d[:, 0:span],
                            16 * (span // nseg), None, OP.subtract)
                    span *= 2
                out_t = outp.tile([128, fd], U16, tag="o")
                nc.vector.tensor_tensor(
                    out_t[:], ones[:], d[:].bitcast(U16),
                    OP.logical_shift_left)
                prev_out = out_t
            nc.scalar.dma_start(y_d[:], prev_out[:])

    nc.compile()
    return nc


_CACHED = {}


def _get_kernel():
    if "k" not in _CACHED:
        _CACHED["k"] = build_kernel()
    return _CACHED["k"]


def _pack(x):
    """f32 one-hot [n,4,256] -> u16 bitmask [n,64]."""
    bits = np.packbits(x != 0, axis=-1, bitorder="little")  # [n,4,32] u8
    return np.ascontiguousarray(bits.reshape(x.shape[0], 128)).view(np.uint16)


def _unpermute(y_raw, n_words=BLOC):
    """Device plane-major [128, fd] -> word-major bitmask [n_words, 64]."""
    s = n_words // 128
    y = y_raw.reshape(128, 16, s, 4)          # p, j, s, seg
    y = y.transpose(0, 2, 3, 1)               # p, s, seg, j
    return np.ascontiguousarray(y.reshape(n_words, 64))


def _unpack(y):
    """u16 bitmask [n,64] -> f32 one-hot [n,4,256]."""
    y8 = np.ascontiguousarray(y).view(np.uint8)             # [n,128]
    bits = np.unpackbits(y8, axis=1, bitorder="little")     # [n,1024]
    return bits.reshape(y.shape[0], 4, 256).astype(np.float32)


def kernel(**inputs):
    a = _pack(np.asarray(inputs["a_bytes"]))
    b = _pack(np.asarray(inputs["b_bytes"]))
    nc = _get_kernel()
    in_maps = [
        {"ab": np.concatenate([a[c * BLOC:(c + 1) * BLOC],
                               b[c * BLOC:(c + 1) * BLOC]])}
        for c in range(NCORES)
    ]
    res = bass_utils.run_bass_kernel_spmd(nc, in_maps, core_ids=list(range(NCORES)))
    out = np.concatenate(
        [_unpermute(res.results[c]["y"]) for c in range(NCORES)], axis=0)
    return _unpack(out)
